# revision 71
# baseline (speedup 1.0000x reference)
"""Trainium2 Bass kernel for MultiLatentAttention (MLA) prefill, 8-way sharded.

v2 strategy (tensor-parallel over heads, restructured for continuous PE):
  - ph1 (q_a | kv_a) projections: the concatenated [w_qa | w_kva[:KVL]]
    output (2048 features = 16 tiles of 128) is split 2 tiles/core; the
    128 rope rows (kd|kr, rotate-half folded on host) are computed by
    every core for a distinct 64-token slice of each chunk (bf16 inputs,
    full PE rate).  All 4 seq-chunks of ph1 stream back-to-back on PE;
    per-chunk AllGathers (X = activations, R = rope, S = sum-of-squares)
    and the RMSNorm scale math (Pool partition_all_reduce + ACT sqrt +
    DVE recip + Pool broadcast) hide under the ph1 stream.
  - phase 2 per chunk: K/V build -> Q build -> attention (scores in
    [k, q] layout, softmax without row-max, multiplicative 0/1 mask on
    the 128-col diagonal band only, matmul widths floored at 256 cols to
    stay at full fp32r rate) -> per-head attn AllGather -> o_proj one
    head-half behind, so AG latency hides under attention PE work.
  - bf16 is used for the big DMA/AG payloads and their matmuls (x, ph1
    weights, A/C activations, q_b/kv_b weights, attn output, o_proj
    weights); the attention core (Q/K/V/E) stays float32r.
"""
import sys

for _p in ("/opt/trn_rl_repo",):
    if _p not in sys.path:
        sys.path.insert(0, _p)

import numpy as np
import ml_dtypes

import concourse.bass as bass
import concourse.bacc as bacc
import concourse.bass_isa as bass_isa
import concourse.mybir as mybir
import concourse.tile as tile
from concourse import bass_utils

F32 = mybir.dt.float32
F32R = mybir.dt.float32r
BF16 = mybir.dt.bfloat16
AF = mybir.ActivationFunctionType
RADD = bass_isa.ReduceOp.add

NCORES = 8
S = 2048; HID = 2048; NH = 16
QL = 1536; KVL = 512
DN = 128; DR = 64; DV = 128; DQK = DN + DR
HPC = NH // NCORES            # heads per core = 2
SCALE = DQK ** -0.5
EPS = 1e-6
QCW = 512                     # q chunk width
NQC = S // QCW                # 4
NKT = S // 128                # 16
RSL = QCW // NCORES           # per-core rope seq slice per chunk = 64

_compiled = None              # cached (nc) program


def _mm(nc, out, lhsT, rhs, start, stop):
    nc.tensor.matmul(out, lhsT, rhs, start=start, stop=stop)


def _ag(nc, fake, rg, in_t, out_t, lat=6):
    """AllGather, or (single-core cost-model mode) a DMA emulation of it.

    lat tunes the emulated latency chain: use the full chain only for
    gathers whose consumers are nearby in the schedule (agT); the ph1
    gathers have tens of us of slack before their consumers, so a short
    chain (lat=1, ~4us) keeps the emulation off the modeled SP queue
    without changing what it predicts.
    """
    if not fake:
        nc.gpsimd.collective_compute(
            "AllGather", mybir.AluOpType.bypass, replica_groups=rg,
            ins=[in_t.opt()], outs=[out_t.opt()])
    else:
        # marker write only: the real collective moves payload over
        # NeuronLink via the collective cores, not the local DMA engines,
        # so a full local copy would double-charge the serial DMA device.
        rows = min(in_t.shape[0], 8)
        cols = min(in_t.shape[1], 64)
        nc.sync.dma_start(out_t[0:rows, 0:cols], in_t[0:rows, 0:cols])
        for _ in range(lat):
            nc.sync.dma_start(out_t[0:1, 0:cols], in_t[0:1, 0:cols])


def _build_body(nc, tc, io, stage=99, fake_coll=False):
    (xTb, xTr, wph1, wrope, wqbx, wkvbk, wkvbv, wosl, cossinT,
     masks, out) = io
    rg = [list(range(NCORES))]
    shared = "Local" if fake_coll else "Shared"

    with tc.tile_pool(name="dram", bufs=1, space="DRAM") as dpool:
        agX_ins = [dpool.tile([256, QCW], BF16, name=f"agX_in{q}")
                   for q in range(NQC)]
        agX_outs = [dpool.tile([2048, QCW], BF16, addr_space=shared,
                               name=f"agX_out{q}") for q in range(NQC)]
        agR_ins = [dpool.tile([128, RSL], F32R, name=f"agR_in{q}")
                   for q in range(NQC)]
        agR_outs = [dpool.tile([NCORES * 128, RSL], F32R, addr_space=shared,
                               name=f"agR_out{q}") for q in range(NQC)]
        agS_ins = [dpool.tile([2, QCW], F32, name=f"agS_in{q}")
                   for q in range(NQC)]
        agS_outs = [dpool.tile([2 * NCORES, QCW], F32, addr_space=shared,
                               name=f"agS_out{q}") for q in range(NQC)]
        agT_ins = [[dpool.tile([DV, QCW], BF16, name=f"agT_in{q}_{h}")
                    for h in range(HPC)] for q in range(NQC)]
        agT_outs = [[dpool.tile([NCORES * DV, QCW], BF16, addr_space=shared,
                                name=f"agT_out{q}_{h}")
                     for h in range(HPC)] for q in range(NQC)]
        skv_dram = dpool.tile([1, S], F32)

        with tc.tile_pool(name="const", bufs=1) as cp, \
             tc.tile_pool(name="wgt", bufs=1) as wg:
            # --- minimal deps for the first matmul go first ------------
            wp_sb = cp.tile([128, NKT, 256], BF16)
            wp_r = wph1.rearrange("(t p) m -> p t m", p=128)
            nc.sync.dma_start(wp_sb[:, 0:8, :], wp_r[:, 0:8, :])
            wr_sb = cp.tile([128, NKT, 128], BF16)
            wr_r = wrope.rearrange("(t p) m -> p t m", p=128)
            nc.sync.dma_start(wr_sb[:, 0:8, :], wr_r[:, 0:8, :])
            xr_sb = cp.tile([128, NKT, NQC, RSL], BF16)
            xr_r = xTr.rearrange("(t p) (c m) -> p t c m", p=128, c=NQC)
            nc.sync.dma_start(xr_sb[:, :, 0, :], xr_r[:, :, 0, :])

            ones_f = cp.tile([128, 1], F32)
            nc.vector.memset(ones_f[:], 1.0)
            ones_r = cp.tile([128, 1], F32R)
            nc.vector.tensor_copy(ones_r[:], ones_f[:])
            cossin = cp.tile([128, S], F32)
            cos_sb = cossin[0:DR]
            sin_sb = cossin[DR:2 * DR]
            mask_sb = cp.tile([128, 256], F32)
            # kpesr: rows 0:64 shared rope key (built per chunk),
            #        rows 64:96 = -sin[0:32], rows 96:128 = sin[32:64]
            kpesr = cp.tile([128, S], F32R)
            kpeT = kpesr[0:DR]
            s_q_b = cp.tile([128, S], F32)
            s_kv_b = cp.tile([128, S], F32)
            skv_col = cp.tile([128, NKT], F32)
            Knope = [cp.tile([DN, S], F32R, name=f"Knope{h}")
                     for h in range(HPC)]
            V_sb = cp.tile([128, NKT, HPC * DV], F32R)
            # warm the ACT tables used later (Square/Sqrt/Exp/Copy)
            warm = cp.tile([1, 4], F32)
            nc.vector.memset(warm[:], 1.0)
            nc.scalar.activation(warm[:, 0:1], warm[:, 0:1], AF.Square)
            nc.scalar.activation(warm[:, 1:2], warm[:, 1:2], AF.Sqrt)
            nc.scalar.activation(warm[:, 2:3], warm[:, 2:3], AF.Exp)
            nc.scalar.copy(warm[:, 3:4], warm[:, 3:4])
            eps_sb = cp.tile([1, 1], F32)
            nc.vector.memset(eps_sb[:], EPS)
            cr0_sb = cp.tile([128, 4, QCW], BF16, name="cr0_sb")

            wqbx_sb = wg.tile([128, 12, HPC * 192], BF16)
            wkbk_sb = wg.tile([128, 4, HPC * DN], BF16)
            wkbv_sb = wg.tile([128, 4, HPC * DV], BF16)
            wo_sb = wg.tile([128, NKT, 256], BF16)

            wqbx_r = wqbx.rearrange("(t p) m -> p t m", p=128)
            wo_r = wosl.rearrange("(t p) m -> p t m", p=128)

            def pre_chunk(ch):
                # the ph1 DMA window is budget-bound by the x stream: only
                # loads that ph1 itself consumes may live here
                if ch == 0:
                    nc.sync.dma_start(wp_sb[:, 8:16, :], wp_r[:, 8:16, :])
                    nc.sync.dma_start(wr_sb[:, 8:16, :], wr_r[:, 8:16, :])
                if ch + 1 < NQC:
                    nc.sync.dma_start(xr_sb[:, :, ch + 1, :],
                                      xr_r[:, :, ch + 1, :])

            # ============ phase 1: sharded projections, 4 chunks =========
            with tc.tile_pool(name="ph1x", bufs=3) as px, \
                 tc.tile_pool(name="ph1t", bufs=2) as pt, \
                 tc.tile_pool(name="ph1n", bufs=1) as pn, \
                 tc.tile_pool(name="ph1p", bufs=2, space="PSUM") as pps:
                def xt_load(ch):
                    c0 = ch * QCW
                    tiles = []
                    for kt8 in range(2):
                        xt = px.tile([128, 8, QCW], BF16, tag="x",
                                     name=f"xt{ch}_{kt8}")
                        nc.sync.dma_start(
                            xt[:], xTb[kt8 * 1024:(kt8 + 1) * 1024,
                                       c0:c0 + QCW].rearrange(
                                           "(eight p) n -> p eight n", p=128))
                        tiles.append(xt)
                    return tiles

                xts = xt_load(0)
                for ch in range(NQC):
                    c0 = ch * QCW
                    pre_chunk(ch)
                    pa = pps.tile([128, QCW], F32, tag="a")
                    pb = pps.tile([128, QCW], F32, tag="b")
                    pr = pps.tile([128, RSL], F32, tag="r")
                    for kt8 in range(2):
                        xt = xts[kt8]
                        for half in range(8):
                            kt = kt8 * 8 + half
                            st, sp = kt == 0, kt == NKT - 1
                            _mm(nc, pa[:], wp_sb[:, kt, 0:128],
                                xt[:, half, :], st, sp)
                            _mm(nc, pb[:], wp_sb[:, kt, 128:256],
                                xt[:, half, :], st, sp)
                            _mm(nc, pr[:], wr_sb[:, kt, :],
                                xr_sb[:, kt, ch, :], st, sp)
                    # prefetch next chunk's x BEFORE the payload DMAs: the
                    # payload configs wait on DVE copies on the SP queue and
                    # would otherwise delay the x stream a full chunk
                    if ch + 1 < NQC:
                        xts = xt_load(ch + 1)
                    aab = pt.tile([128, 2, QCW], BF16, tag="aab")
                    ar = pt.tile([128, RSL], F32R, tag="ar")
                    nc.vector.tensor_copy(aab[:, 0, :], pa[:])
                    nc.vector.tensor_copy(aab[:, 1, :], pb[:])
                    nc.vector.tensor_copy(ar[:], pr[:])
                    sqa = pt.tile([128, QCW], F32, tag="sqa", bufs=1)
                    sqb = pt.tile([128, QCW], F32, tag="sqb", bufs=1)
                    nc.scalar.activation(sqa[:], pa[:], AF.Square)
                    nc.scalar.activation(sqb[:], pb[:], AF.Square)
                    nc.sync.dma_start(
                        agX_ins[ch][:].rearrange("(two p) n -> p two n",
                                                 p=128), aab[:])
                    nc.scalar.dma_start(agR_ins[ch][:], ar[:])
                    _ag(nc, fake_coll, rg, agX_ins[ch], agX_outs[ch], lat=0)
                    _ag(nc, fake_coll, rg, agR_ins[ch], agR_outs[ch], lat=0)
                    # per-tile sum of squares on Pool (partition reduce)
                    ra = pt.tile([128, QCW], F32, tag="ra", bufs=1)
                    rb = pt.tile([128, QCW], F32, tag="rb", bufs=1)
                    nc.gpsimd.partition_all_reduce(ra[:], sqa[:], 128, RADD)
                    nc.gpsimd.partition_all_reduce(rb[:], sqb[:], 128, RADD)
                    nc.scalar.dma_start(agS_ins[ch][0:1, :], ra[0:1, :])
                    nc.scalar.dma_start(agS_ins[ch][1:2, :], rb[0:1, :])
                    _ag(nc, fake_coll, rg, agS_ins[ch], agS_outs[ch], lat=0)
                    # ---- norm scales for this chunk (non-PE engines) ----
                    # gathered row order = global tile order: rows 0:12 are
                    # the 12 q_a tiles, rows 12:16 the 4 kv tiles
                    s12 = pn.tile([12, QCW], F32, tag="s12")
                    s4 = pn.tile([4, QCW], F32, tag="s4")
                    nc.gpsimd.dma_start(s12[:], agS_outs[ch][0:12, :])
                    nc.gpsimd.dma_start(s4[:], agS_outs[ch][12:16, :])
                    sq12 = pn.tile([12, QCW], F32, tag="sq12")
                    sk4 = pn.tile([4, QCW], F32, tag="sk4")
                    nc.gpsimd.partition_all_reduce(sq12[:], s12[:], 12, RADD)
                    nc.gpsimd.partition_all_reduce(sk4[:], s4[:], 4, RADD)
                    tq = pn.tile([1, QCW], F32, tag="tq")
                    tk = pn.tile([1, QCW], F32, tag="tk")
                    nc.scalar.activation(tq[:], sq12[0:1, :], AF.Sqrt,
                                         bias=eps_sb[:], scale=1.0 / QL)
                    nc.scalar.activation(tk[:], sk4[0:1, :], AF.Sqrt,
                                         bias=eps_sb[:], scale=1.0 / KVL)
                    srow = pn.tile([1, QCW], F32, tag="srow")
                    krow = pn.tile([1, QCW], F32, tag="krow")
                    nc.vector.reciprocal(srow[:], tq[:])
                    nc.vector.reciprocal(krow[:], tk[:])
                    nc.gpsimd.partition_broadcast(s_q_b[:, c0:c0 + QCW],
                                                  srow[:])
                    nc.gpsimd.partition_broadcast(s_kv_b[:, c0:c0 + QCW],
                                                  krow[:])
                    nc.gpsimd.dma_start(skv_dram[:, c0:c0 + QCW], krow[:])
                    nc.gpsimd.dma_start(
                        skv_col[:, 4 * ch:4 * ch + 4],
                        skv_dram[:, c0:c0 + QCW].rearrange(
                            "a (t p) -> (a p) t", p=128))

            if stage < 2:
                z = cp.tile([128, S], F32, name="zdump")
                nc.vector.memset(z[:], 0.0)
                nc.sync.dma_start(out[0:128, :], z[:])
                nc.sync.dma_start(out[128:256, :], z[:])
                return

            # ============ phase 2: KV/Q build, attention, o_proj =========
            with tc.tile_pool(name="cr", bufs=2) as crp, \
                 tc.tile_pool(name="atp", bufs=6) as atp, \
                 tc.tile_pool(name="qp", bufs=2) as qp, \
                 tc.tile_pool(name="bqt", bufs=1) as bqt, \
                 tc.tile_pool(name="ae", bufs=4) as ae, \
                 tc.tile_pool(name="rtp", bufs=6) as rtp, \
                 tc.tile_pool(name="mg", bufs=1) as mg, \
                 tc.tile_pool(name="obp", bufs=1) as obp, \
                 tc.tile_pool(name="stp", bufs=3, space="PSUM") as stp, \
                 tc.tile_pool(name="pvp", bufs=2, space="PSUM") as pvp, \
                 tc.tile_pool(name="dnp", bufs=1, space="PSUM") as dnp, \
                 tc.tile_pool(name="opp", bufs=1, space="PSUM") as opp:
                # phase-2 consts + weights stream in under the early
                # phase-2 compute (ordered by first consumer)
                nc.scalar.dma_start(cossin[:], cossinT[:])
                nc.scalar.dma_start(
                    wkbk_sb[:], wkvbk.rearrange("(t p) m -> p t m", p=128))
                nc.scalar.dma_start(
                    wkbv_sb[:], wkvbv.rearrange("(t p) m -> p t m", p=128))
                nc.vector.tensor_scalar_mul(kpesr[DR:DR + 32, :],
                                            sin_sb[0:32, :], -1.0)
                nc.vector.tensor_copy(kpesr[DR + 32:DR + 64, :],
                                      sin_sb[32:64, :])
                nc.scalar.dma_start(wqbx_sb[:, 0:6, :], wqbx_r[:, 0:6, :])
                nc.scalar.dma_start(wqbx_sb[:, 6:12, :], wqbx_r[:, 6:12, :])
                nc.scalar.dma_start(mask_sb[:], masks[:])
                nc.scalar.dma_start(wo_sb[:, 0:8, :], wo_r[:, 0:8, :])
                nc.scalar.dma_start(wo_sb[:, 8:16, :], wo_r[:, 8:16, :])

                def kv_chunk(ch, pre=None):
                    c0 = ch * QCW
                    if pre is None:
                        cr = crp.tile([128, 4, QCW], BF16, tag="cr")
                        nc.sync.dma_start(
                            cr[:], agX_outs[ch][1536:2048, :].rearrange(
                                "(t p) m -> p t m", p=128))
                    else:
                        cr = pre
                    kdr3 = crp.tile([128, NCORES, RSL], F32R, tag="kdr")
                    nc.sync.dma_start(
                        kdr3[:], agR_outs[ch][:].rearrange(
                            "(c p) m -> p c m", p=128))
                    kdr = kdr3[:].rearrange("p c m -> p (c m)")
                    # shared rope key for this chunk (unnormalized)
                    t2 = crp.tile([DR, QCW], F32, tag="t2")
                    nc.vector.tensor_mul(kpeT[:, c0:c0 + QCW], kdr[0:DR],
                                         cos_sb[:, c0:c0 + QCW])
                    nc.vector.tensor_mul(t2[:], kdr[DR:2 * DR],
                                         sin_sb[:, c0:c0 + QCW])
                    nc.vector.tensor_add(kpeT[:, c0:c0 + QCW],
                                         kpeT[:, c0:c0 + QCW], t2[:])
                    for h in range(HPC):
                        pk = stp.tile([128, QCW], F32, tag="st")
                        for lt in range(4):
                            _mm(nc, pk[:], wkbk_sb[:, lt, h * DN:(h + 1) * DN],
                                cr[:, lt, :], lt == 0, lt == 3)
                        nc.vector.tensor_mul(Knope[h][:, c0:c0 + QCW], pk[:],
                                             s_kv_b[:, c0:c0 + QCW])
                    for sl in range(4):
                        st = ch * 4 + sl
                        pv = stp.tile([128, QCW], F32, tag="st")
                        for lt in range(4):
                            _mm(nc, pv[:, 0:HPC * DV],
                                cr[:, lt, sl * 128:(sl + 1) * 128],
                                wkbv_sb[:, lt, :], lt == 0, lt == 3)
                        nc.scalar.copy(V_sb[:, st, :], pv[:, 0:HPC * DV])
                        nc.vector.tensor_scalar_mul(V_sb[:, st, :],
                                                    V_sb[:, st, :],
                                                    skv_col[:, st:st + 1])

                def q_chunk(ch):
                    c0 = ch * QCW
                    ats = []
                    agX_r = agX_outs[ch][0:1536, :].rearrange(
                        "(t p) m -> p t m", p=128)
                    for g in range(4):
                        at = atp.tile([128, 3, QCW], BF16, tag="at")
                        nc.sync.dma_start(at[:], agX_r[:, 3 * g:3 * g + 3, :])
                        ats.append(at)
                    Qn = [qp.tile([128, QCW], F32R, tag=f"qn{h}",
                                  name=f"Qn{h}_{ch}") for h in range(HPC)]
                    Qpe = [qp.tile([DR, QCW], F32R, tag=f"qp{h}",
                                   name=f"Qpe{h}_{ch}") for h in range(HPC)]
                    # pe columns first so the rope DVE chain runs under the
                    # two nope matmul groups
                    for mt in (2, 0, 1):
                        pq = stp.tile([128, QCW], F32, tag="st")
                        for kt in range(12):
                            _mm(nc, pq[:],
                                wqbx_sb[:, kt, mt * 128:(mt + 1) * 128],
                                ats[kt // 3][:, kt % 3, :],
                                kt == 0, kt == 11)
                        if mt < 2:
                            nc.vector.tensor_mul(Qn[mt][:], pq[:],
                                                 s_q_b[:, c0:c0 + QCW])
                        else:
                            for h in range(HPC):
                                u1 = bqt.tile([DR, QCW], F32, tag="u1")
                                t2 = bqt.tile([DR, QCW], F32, tag="t2")
                                nc.vector.tensor_mul(
                                    u1[:], pq[DR * h:DR * h + DR, :],
                                    cos_sb[:, c0:c0 + QCW])
                                nc.vector.tensor_mul(
                                    t2[0:32, :], pq[DR * h + 32:DR * h + 64, :],
                                    kpesr[DR:DR + 32, c0:c0 + QCW])
                                nc.vector.tensor_mul(
                                    t2[32:64, :], pq[DR * h:DR * h + 32, :],
                                    kpesr[DR + 32:DR + 64, c0:c0 + QCW])
                                nc.vector.tensor_add(u1[:], u1[:], t2[:])
                                nc.vector.tensor_mul(
                                    Qpe[h][:], u1[:],
                                    s_q_b[0:DR, c0:c0 + QCW])
                    return Qn, Qpe

                def attn_head(qc, h, Qn, Qpe):
                    c0 = qc * QCW
                    nk = 4 * qc + 4
                    pden = dnp.tile([1, QCW], F32, tag="dn")
                    ppv = pvp.tile([DV, QCW], F32, tag="pv")
                    Es = {}

                    def scores(kt):
                        t = kt - 4 * qc
                        off = min(128 * t, 256) if t > 0 else 0
                        ps = stp.tile([128, QCW], F32, tag="st")
                        _mm(nc, ps[:, off:],
                            Knope[h][:, kt * 128:(kt + 1) * 128],
                            Qn[h][:, off:], True, False)
                        _mm(nc, ps[:, off:], kpeT[:, kt * 128:(kt + 1) * 128],
                            Qpe[h][:, off:], False, True)
                        E = ae.tile([128, QCW], F32R, tag="e")
                        nc.scalar.activation(E[:, off:], ps[:, off:], AF.Exp,
                                             scale=SCALE)
                        if t >= 0:
                            if t < 3:
                                nc.vector.tensor_mul(
                                    E[:, off:off + 128], E[:, off:off + 128],
                                    mask_sb[:, 128:256])
                            else:
                                nc.vector.tensor_mul(
                                    E[:, 256:512], E[:, 256:512],
                                    mask_sb[:, 0:256])
                        Es[kt] = (E, off)

                    def accum(kt):
                        E, off = Es.pop(kt)
                        _mm(nc, pden[:, off:], ones_r[:], E[:, off:],
                            kt == 0, kt == nk - 1)
                        _mm(nc, ppv[:, off:], V_sb[:, kt, h * DV:(h + 1) * DV],
                            E[:, off:], kt == 0, kt == nk - 1)

                    # lag-2 software pipeline: den/PV of tile kt-2 issue
                    # after the scores of tile kt, so PE never waits on the
                    # ACT exp / DVE mask chain
                    for kt in range(nk):
                        scores(kt)
                        if kt >= 2:
                            accum(kt - 2)
                    accum(nk - 2)
                    accum(nk - 1)
                    recd = mg.tile([1, QCW], F32, tag="rd", bufs=2)
                    nc.vector.reciprocal(recd[:], pden[:])
                    recb = mg.tile([128, QCW], F32, tag="rb", bufs=1)
                    nc.gpsimd.partition_broadcast(recb[:], recd[:])
                    ao = mg.tile([DV, QCW], BF16, tag="ao", bufs=2)
                    nc.vector.tensor_mul(ao[:], ppv[:], recb[0:DV, :])
                    nc.sync.dma_start(agT_ins[qc][h][:], ao[:])
                    _ag(nc, fake_coll, rg, agT_ins[qc][h], agT_outs[qc][h])

                def rt_load(qc, h):
                    rt = rtp.tile([128, 4, QCW], BF16, tag="rt",
                                  name=f"rt{qc}_{h}")
                    nc.sync.dma_start(
                        rt[:], agT_outs[qc][h][0:512, :].rearrange(
                            "(t p) m -> p t m", p=128))
                    rt2 = rtp.tile([128, 4, QCW], BF16, tag="rt",
                                   name=f"rt2{qc}_{h}")
                    nc.sync.dma_start(
                        rt2[:], agT_outs[qc][h][512:1024, :].rearrange(
                            "(t p) m -> p t m", p=128))
                    return rt, rt2

                def oproj_mm(qc, h, po, rts):
                    rt, rt2 = rts
                    for r in range(8):
                        kt = 8 * h + r
                        src = rt if r < 4 else rt2
                        for m in range(2):
                            _mm(nc, po[m][:],
                                wo_sb[:, kt, m * 128:(m + 1) * 128],
                                src[:, r % 4, :], kt == 0, kt == NKT - 1)
                    if h == HPC - 1:
                        c0 = qc * QCW
                        for m in range(2):
                            ob = obp.tile([128, QCW], F32, tag="ob")
                            nc.scalar.copy(ob[:], po[m][:])
                            nc.sync.dma_start(
                                out[m * 128:(m + 1) * 128, c0:c0 + QCW],
                                ob[:])

                def oproj_half(qc, h, po):
                    oproj_mm(qc, h, po, rt_load(qc, h))

                po_live = {}

                def oproj_start(qc):
                    po_live[qc] = [opp.tile([128, QCW], F32, tag=f"po{m}",
                                            name=f"po{m}_{qc}")
                                   for m in range(2)]

                # pipeline: attention heads stream; o_proj halves are
                # interleaved behind, but ~14us of o_proj work is held back
                # to the end so the last attn AllGather's latency hides
                # under PE work instead of exposing a tail stall.  The held
                # back halves' rt readbacks are issued as soon as their AG
                # lands so the transfers don't bunch up in the tail.
                nc.sync.dma_start(
                    cr0_sb[:], agX_outs[0][1536:2048, :].rearrange(
                        "(t p) m -> p t m", p=128))
                for qc in range(NQC):
                    kv_chunk(qc, pre=cr0_sb if qc == 0 else None)
                    Qn, Qpe = q_chunk(qc)
                    for h in range(HPC):
                        attn_head(qc, h, Qn, Qpe)
                        if qc == 1:
                            if h == 0:
                                oproj_start(0)
                            oproj_half(0, h, po_live[0])
                        elif qc == 2 and h == 0:
                            oproj_start(1)
                            oproj_half(1, 0, po_live[1])
                        elif qc == 3 and h == 0:
                            rts11 = rt_load(1, 1)
                        elif qc == 3 and h == 1:
                            rts20 = rt_load(2, 0)
                            rts21 = rt_load(2, 1)
                oproj_mm(1, 1, po_live[1], rts11)
                rts30 = rt_load(3, 0)
                oproj_start(2)
                oproj_mm(2, 0, po_live[2], rts20)
                oproj_mm(2, 1, po_live[2], rts21)
                rts31 = rt_load(3, 1)
                oproj_start(3)
                oproj_mm(3, 0, po_live[3], rts30)
                oproj_mm(3, 1, po_live[3], rts31)


def build_program(stage=99, n_devices=NCORES, repeat=1):
    nc = bacc.Bacc("TRN2", target_bir_lowering=False, debug=False,
                   enable_asserts=True, num_devices=n_devices)

    def din(name, shape, dt=F32):
        return nc.dram_tensor(name, shape, dt, kind="ExternalInput").ap()

    io = (
        din("xTb", [HID, S], BF16),
        din("xTr", [HID, NQC * RSL], BF16),
        din("wph1", [HID, 256], BF16),
        din("wrope", [HID, 128], BF16),
        din("wqbx", [QL, HPC * 192], BF16),
        din("wkvbk", [KVL, HPC * DN], BF16),
        din("wkvbv", [KVL, HPC * DV], BF16),
        din("wosl", [NH * DV, 256], BF16),
        din("cossinT", [128, S]),
        din("masks", [128, 256]),
        nc.dram_tensor("out", [256, S], F32, kind="ExternalOutput").ap(),
    )
    with tile.TileContext(nc) as tc:
        for _r in range(repeat):
            _build_body(nc, tc, io, stage, fake_coll=(n_devices == 1))
    nc.compile()
    return nc


# ---------------- host-side prep ------------------------------------------

_PERM = [2 * (j % 32) + j // 32 for j in range(DR)]


def _fold_pe(wpe):
    """Fold rope de-interleave + rotate-half into weight columns.

    wpe: [..., 64].  Returns (deinterleaved, rotated) column variants such
    that rope(x @ wpe) == (x @ wd) * cos + (x @ wr) * sin.
    """
    wd = wpe[:, _PERM]
    wr = np.concatenate([-wd[:, 32:], wd[:, :32]], axis=1)
    return wd, wr


def _wosl_perm(w_o, c):
    """w_o rows reordered to match the per-head AllGather layout:
    kt 0-7 are each core's head-0 (global heads 0,2,..14), kt 8-15 the
    head-1s (1,3,..15)."""
    rows = []
    for h in list(range(0, NH, 2)) + list(range(1, NH, 2)):
        rows.append(w_o[h * DV:(h + 1) * DV])
    return np.ascontiguousarray(
        np.concatenate(rows, axis=0)[:, c * 256:(c + 1) * 256])


def host_prep(hidden_states, cos, sin, w_qa, g_qa, w_qb, w_kva, g_kva, w_kvb, w_o):
    """Build the 8 per-core input dicts."""
    f32 = np.float32
    bf16 = ml_dtypes.bfloat16
    xT = np.ascontiguousarray(np.asarray(hidden_states, f32)[0].T)
    xTb = xT.astype(bf16)
    w_qb2 = np.asarray(g_qa, f32)[:, None] * np.asarray(w_qb, f32)
    w_kvb2 = np.asarray(g_kva, f32)[:, None] * np.asarray(w_kvb, f32)
    w_qa = np.asarray(w_qa, f32)
    w_kva = np.asarray(w_kva, f32)
    w_o = np.asarray(w_o, f32)

    kd, kr = _fold_pe(w_kva[:, KVL:KVL + DR])
    wrope = np.concatenate([kd, kr], axis=1).astype(bf16)  # [HID, 128]
    # [w_qa | w_kva[:, :KVL]] : [HID, 2048] -> per-core 256-col slice
    wph1_full = np.concatenate([w_qa, w_kva[:, :KVL]], axis=1).astype(bf16)

    # per-head-pair q_b blocks: [nope_h0(128) | nope_h1(128) | pe_h0(64)+pe_h1(64)]
    wqbx_cores = []
    for c in range(NCORES):
        h0 = c * HPC
        cols = []
        pe_cols = []
        for h in range(HPC):
            base = (h0 + h) * DQK
            cols.append(w_qb2[:, base:base + DN])
            wd, _ = _fold_pe(w_qb2[:, base + DN:base + DQK])
            pe_cols.append(wd)
        wqbx_cores.append(np.ascontiguousarray(
            np.concatenate(cols + pe_cols, axis=1)).astype(bf16))

    cossinT = np.ascontiguousarray(np.concatenate(
        [np.asarray(cos, f32)[0].T, np.asarray(sin, f32)[0].T], axis=0))

    masks = np.zeros((128, 256), f32)
    r = np.arange(128)[:, None]
    j = np.arange(128)[None, :]
    masks[:, 128:256] = (r <= j).astype(f32)

    in_maps = []
    for c in range(NCORES):
        h0 = c * HPC
        kcols = []
        vcols = []
        for h in range(HPC):
            base = (h0 + h) * (DN + DV)
            kcols.append(w_kvb2[:, base:base + DN])
            vcols.append(w_kvb2[:, base + DN:base + DN + DV])
        # rope x slice: for each chunk, tokens [ch*512 + c*64, +64)
        xtr = np.concatenate(
            [xT[:, ch * QCW + c * RSL:ch * QCW + (c + 1) * RSL]
             for ch in range(NQC)], axis=1).astype(bf16)
        in_maps.append({
            "xTb": xTb,
            "xTr": np.ascontiguousarray(xtr),
            "wph1": np.ascontiguousarray(wph1_full[:, c * 256:(c + 1) * 256]),
            "wrope": wrope,
            "wqbx": wqbx_cores[c],
            "wkvbk": np.ascontiguousarray(
                np.concatenate(kcols, axis=1)).astype(bf16),
            "wkvbv": np.ascontiguousarray(
                np.concatenate(vcols, axis=1)).astype(bf16),
            "wosl": _wosl_perm(w_o, c).astype(bf16),
            "cossinT": cossinT,
            "masks": masks,
        })
    return in_maps


def kernel(**inputs):
    global _compiled
    if _compiled is None:
        _compiled = build_program()
    nc = _compiled
    in_maps = host_prep(**inputs)
    res = bass_utils.run_bass_kernel_spmd(
        nc, in_maps, core_ids=list(range(NCORES)))
    kernel.last_results = res
    cols = [np.asarray(res.results[c]["out"], np.float32).T
            for c in range(NCORES)]
    return np.concatenate(cols, axis=1)[None]
